# revision 11
# baseline (speedup 1.0000x reference)
"""ConstituencyTreeLSTM Trainium2 kernel.

Strategy:
  - Data-parallel over the B=256 batch across 8 NeuronCores (32 rows/core).
  - The tree is a complete heap (node i has children 2i+1, 2i+2), so the
    sequential scan is reorganized into level-parallel phases:
      leaves (nodes 128..255) -> node 127 -> level 6 (63..126) -> ... -> root.
  - Everything on-device lives in a "feature-on-partitions, (node,batch) rows
    on free axis" layout, so matmul outputs (PSUM, [out_dim, rows]) are already
    in the layout needed to feed the next level's matmul. No transposes.
  - One fused bf16 weight matrix W_big [1536, 2560]:
      rows:  0:512 x | 512:1024 hL | 1024:1536 hR
      cols:  0:1536 iou | 1536:2048 fL-pre | 2048:2560 fR-pre
    Zero blocks (hR->fL, hL->fR) are skipped by not emitting those k-tiles.
  - Per-node-type biases (2-child / leaf / 1-child) folded host-side and
    applied inside the PSUM-evacuating activation (sigmoid/tanh).
  - Children h/c are stored parity-split (HL/HR, CL/CR indexed by parent), so
    every gather/scatter DMA is contiguous.
"""

import sys

sys.path.insert(0, "/opt/trn_rl_repo")

import numpy as np
import ml_dtypes

import concourse.bass as bass  # noqa: F401  (import registers bass machinery)
import concourse.mybir as mybir
import concourse.tile as tile
from concourse import bacc
from concourse.bass_utils import run_bass_kernel_spmd

BF16 = ml_dtypes.bfloat16
NCORES = 8
B, N, D = 256, 256, 512
BC = B // NCORES  # batch rows per core
KT_X, KT_HL, KT_HR = range(0, 4), range(4, 8), range(8, 12)
NJ = 20  # output j-tiles: 12 iou + 4 fL + 4 fR

_compiled = {}


def _build_bass(reps=1):
    nc = bacc.Bacc("TRN2", target_bir_lowering=False, debug=False, num_devices=NCORES)

    f32 = mybir.dt.float32
    bf16 = mybir.dt.bfloat16

    xt = nc.dram_tensor("xt", [N, D, BC], bf16, kind="ExternalInput")
    w = nc.dram_tensor("w", [12, 128, 128 * NJ], bf16, kind="ExternalInput")
    b2_d = nc.dram_tensor("b2", [128, NJ], f32, kind="ExternalInput")
    bleaf_d = nc.dram_tensor("bleaf", [128, NJ], f32, kind="ExternalInput")
    b1_d = nc.dram_tensor("b1", [128, NJ], f32, kind="ExternalInput")

    # children h/c keyed by parent index t: HL[t] = h(2t+1), HR[t] = h(2t+2)
    HL = nc.dram_tensor("HLbuf", [128, D, BC], bf16)
    HR = nc.dram_tensor("HRbuf", [128, D, BC], bf16)
    CL = nc.dram_tensor("CLbuf", [128, D, BC], bf16)
    CR = nc.dram_tensor("CRbuf", [128, D, BC], bf16)

    c0t = nc.dram_tensor("c0t", [D, BC], f32, kind="ExternalOutput")
    h0t = nc.dram_tensor("h0t", [D, BC], f32, kind="ExternalOutput")

    # all views are [partition, node, ktile, batch]; (node, ktile) merge on DMA
    xt_r = xt.ap().rearrange("n (kt p) b -> p n kt b", p=128)
    HL_r = HL.ap().rearrange("t (kt p) b -> p t kt b", p=128)
    HR_r = HR.ap().rearrange("t (kt p) b -> p t kt b", p=128)
    CL_r = CL.ap().rearrange("t (kt p) b -> p t kt b", p=128)
    CR_r = CR.ap().rearrange("t (kt p) b -> p t kt b", p=128)
    c0t_r = c0t.ap().rearrange("(kt p) b -> p kt b", p=128)
    h0t_r = h0t.ap().rearrange("(kt p) b -> p kt b", p=128)

    with tile.TileContext(nc) as tc:
        import contextlib

        ctx = contextlib.ExitStack()
        with ctx:
            wpool = ctx.enter_context(tc.tile_pool(name="wpool", bufs=1))
            inpool = ctx.enter_context(tc.tile_pool(name="inpool", bufs=2))
            gpool = ctx.enter_context(tc.tile_pool(name="gpool", bufs=2))
            epool = ctx.enter_context(tc.tile_pool(name="epool", bufs=2))
            pspool = ctx.enter_context(tc.tile_pool(name="ps", bufs=8, space="PSUM"))

            w_sb = wpool.tile([128, 12, 128 * NJ], bf16)
            for kt in range(12):
                nc.sync.dma_start(out=w_sb[:, kt, :], in_=w.ap()[kt])
            b2_sb = wpool.tile([128, NJ], f32, name="b2sb")
            bleaf_sb = wpool.tile([128, NJ], f32, name="bleafsb")
            b1_sb = wpool.tile([128, NJ], f32, name="b1sb")
            nc.sync.dma_start(out=b2_sb[:], in_=b2_d.ap()[:])
            nc.sync.dma_start(out=bleaf_sb[:], in_=bleaf_d.ap()[:])
            nc.sync.dma_start(out=b1_sb[:], in_=b1_d.ap()[:])

            def process(nodes, has_l, has_r, bias_sb, to_out):
                """Compute (c, h) for `nodes` (a range), all at the same tree depth."""
                for a in range(nodes.start, nodes.stop, 16):
                    b_ = min(a + 16, nodes.stop)
                    k = b_ - a  # nodes in this chunk
                    n = k * BC  # matmul rows in this chunk
                    dt_g = f32 if to_out else bf16

                    xt_t = inpool.tile([128, k, 4, BC], bf16, name="xt_t")
                    nc.sync.dma_start(out=xt_t[:], in_=xt_r[:, a:b_, :, :])
                    if has_l:
                        hl_t = inpool.tile([128, k, 4, BC], bf16, name="hl_t")
                        cl_t = inpool.tile([128, k, 4, BC], bf16, name="cl_t")
                        nc.sync.dma_start(out=hl_t[:], in_=HL_r[:, a:b_, :, :])
                        nc.sync.dma_start(out=cl_t[:], in_=CL_r[:, a:b_, :, :])
                    if has_r:
                        hr_t = inpool.tile([128, k, 4, BC], bf16, name="hr_t")
                        cr_t = inpool.tile([128, k, 4, BC], bf16, name="cr_t")
                        nc.sync.dma_start(out=hr_t[:], in_=HR_r[:, a:b_, :, :])
                        nc.sync.dma_start(out=cr_t[:], in_=CR_r[:, a:b_, :, :])

                    g_i = gpool.tile([128, k, 4, BC], dt_g, name="g_i")
                    g_o = gpool.tile([128, k, 4, BC], dt_g, name="g_o")
                    g_u = gpool.tile([128, k, 4, BC], dt_g, name="g_u")
                    if has_l:
                        g_fl = gpool.tile([128, k, 4, BC], dt_g, name="g_fl")
                    if has_r:
                        g_fr = gpool.tile([128, k, 4, BC], dt_g, name="g_fr")

                    js = list(range(12))
                    if has_l:
                        js += list(range(12, 16))
                    if has_r:
                        js += list(range(16, 20))

                    for j in js:
                        if j < 12:
                            kts = list(KT_X)
                            if has_l:
                                kts += list(KT_HL)
                            if has_r:
                                kts += list(KT_HR)
                        elif j < 16:
                            kts = list(KT_X) + list(KT_HL)
                        else:
                            kts = list(KT_X) + list(KT_HR)

                        ps = pspool.tile([128, k, BC], f32, name="ps")
                        for i, kt in enumerate(kts):
                            if kt < 4:
                                rhs = xt_t[:, :, kt, :]
                            elif kt < 8:
                                rhs = hl_t[:, :, kt - 4, :]
                            else:
                                rhs = hr_t[:, :, kt - 8, :]
                            nc.tensor.matmul(
                                ps[:],
                                w_sb[:, kt, j * 128 : (j + 1) * 128],
                                rhs,
                                start=(i == 0),
                                stop=(i == len(kts) - 1),
                            )
                        func = (
                            mybir.ActivationFunctionType.Tanh
                            if 8 <= j < 12
                            else mybir.ActivationFunctionType.Sigmoid
                        )
                        if j < 4:
                            dst = g_i[:, :, j, :]
                        elif j < 8:
                            dst = g_o[:, :, j - 4, :]
                        elif j < 12:
                            dst = g_u[:, :, j - 8, :]
                        elif j < 16:
                            dst = g_fl[:, :, j - 12, :]
                        else:
                            dst = g_fr[:, :, j - 16, :]
                        nc.scalar.activation(
                            out=dst,
                            in_=ps[:],
                            func=func,
                            bias=bias_sb[:, j : j + 1],
                            scale=1.0,
                        )

                    # c = i*u (+ fl*cl) (+ fr*cr);  h = o * tanh(c)
                    c_t = epool.tile([128, k, 4, BC], dt_g, name="c_t")
                    nc.vector.tensor_mul(c_t[:], g_i[:], g_u[:])
                    if has_l:
                        t1 = epool.tile([128, k, 4, BC], dt_g, name="t1")
                        nc.vector.tensor_mul(t1[:], g_fl[:], cl_t[:])
                        nc.vector.tensor_add(c_t[:], c_t[:], t1[:])
                    if has_r:
                        t2 = epool.tile([128, k, 4, BC], dt_g, name="t2")
                        nc.vector.tensor_mul(t2[:], g_fr[:], cr_t[:])
                        nc.vector.tensor_add(c_t[:], c_t[:], t2[:])
                    tc_t = epool.tile([128, k, 4, BC], dt_g, name="tc_t")
                    nc.scalar.activation(
                        out=tc_t[:], in_=c_t[:], func=mybir.ActivationFunctionType.Tanh
                    )
                    h_t = epool.tile([128, k, 4, BC], dt_g, name="h_t")
                    nc.vector.tensor_mul(h_t[:], g_o[:], tc_t[:])

                    if to_out:
                        nc.sync.dma_start(out=c0t_r[:], in_=c_t[:, 0, :, :])
                        nc.sync.dma_start(out=h0t_r[:], in_=h_t[:, 0, :, :])
                    else:
                        # node t -> HL[(t-1)//2] if t odd else HR[t//2 - 1]
                        odd0 = 0 if a % 2 == 1 else 1  # offset of first odd node
                        even0 = 1 - odd0
                        odds = range(a + odd0, b_, 2)
                        evens = range(a + even0, b_, 2)
                        for kt in range(4):
                            if len(odds):
                                lo = (odds[0] - 1) // 2
                                nc.sync.dma_start(
                                    out=HL_r[:, lo : lo + len(odds), kt, :],
                                    in_=h_t[:, odd0::2, kt, :],
                                )
                                nc.sync.dma_start(
                                    out=CL_r[:, lo : lo + len(odds), kt, :],
                                    in_=c_t[:, odd0::2, kt, :],
                                )
                            if len(evens):
                                ro = evens[0] // 2 - 1
                                nc.sync.dma_start(
                                    out=HR_r[:, ro : ro + len(evens), kt, :],
                                    in_=h_t[:, even0::2, kt, :],
                                )
                                nc.sync.dma_start(
                                    out=CR_r[:, ro : ro + len(evens), kt, :],
                                    in_=c_t[:, even0::2, kt, :],
                                )

            for _rep in range(reps):
                # leaves: nodes 128..255 (no children)
                process(range(128, 256), False, False, bleaf_sb, False)
                # node 127: left child only (node 255)
                process(range(127, 128), True, False, b1_sb, False)
                # levels 6..1: two children each
                for lvl in range(6, 0, -1):
                    process(
                        range(2**lvl - 1, 2 ** (lvl + 1) - 1), True, True, b2_sb, False
                    )
                # root
                process(range(0, 1), True, True, b2_sb, True)

    nc.compile()
    return nc


def _expected_tree():
    left = np.array([2 * i + 1 if 2 * i + 1 < N else 0 for i in range(N)], np.int32)
    right = np.array([2 * i + 2 if 2 * i + 2 < N else 0 for i in range(N)], np.int32)
    nch = np.array(
        [int(2 * i + 1 < N) + int(2 * i + 2 < N) for i in range(N)], np.int32
    )
    return left, right, nch


def kernel(
    inputs,
    W_ioux, b_ioux, W_iouh, b_iouh, W_iouhL, b_iouhL, W_iouhR, b_iouhR,
    W_fx, b_fx, W_fh, b_fh, W_fhL, b_fhL, W_fhR, b_fhR,
    left_idx, right_idx, num_children,
):
    el, er, en = _expected_tree()
    assert np.array_equal(np.asarray(left_idx), el), "unexpected tree structure"
    assert np.array_equal(np.asarray(right_idx), er), "unexpected tree structure"
    assert np.array_equal(np.asarray(num_children), en), "unexpected tree structure"

    inputs = np.asarray(inputs, np.float32)

    # W_big [1536, 2560]
    w_big = np.zeros((1536, 2560), np.float32)
    w_big[0:512, 0:1536] = np.asarray(W_ioux, np.float32).T
    w_big[0:512, 1536:2048] = np.asarray(W_fx, np.float32).T
    w_big[0:512, 2048:2560] = np.asarray(W_fx, np.float32).T
    w_big[512:1024, 0:1536] = np.asarray(W_iouhL, np.float32).T
    w_big[512:1024, 1536:2048] = np.asarray(W_fhL, np.float32).T
    w_big[1024:1536, 0:1536] = np.asarray(W_iouhR, np.float32).T
    w_big[1024:1536, 2048:2560] = np.asarray(W_fhR, np.float32).T
    w_np = np.ascontiguousarray(w_big.reshape(12, 128, 2560)).astype(BF16)

    def pack_bias(vec):
        return np.ascontiguousarray(np.asarray(vec, np.float32).reshape(NJ, 128).T)

    b_ioux = np.asarray(b_ioux, np.float32)
    b_iouh = np.asarray(b_iouh, np.float32)
    b_iouhL = np.asarray(b_iouhL, np.float32)
    b_iouhR = np.asarray(b_iouhR, np.float32)
    b_fx = np.asarray(b_fx, np.float32)
    b_fhL = np.asarray(b_fhL, np.float32)
    b_fhR = np.asarray(b_fhR, np.float32)

    b2 = pack_bias(
        np.concatenate([b_ioux + b_iouhL + b_iouhR, b_fx + b_fhL, b_fx + b_fhR])
    )
    bleaf = pack_bias(np.concatenate([b_ioux + b_iouh, np.zeros(1024, np.float32)]))
    b1 = pack_bias(
        np.concatenate([b_ioux + b_iouhL, b_fx + b_fhL, np.zeros(512, np.float32)])
    )

    if "nc" not in _compiled:
        _compiled["nc"] = _build_bass()
    nc = _compiled["nc"]

    in_maps = []
    for c in range(NCORES):
        xc = inputs[c * BC : (c + 1) * BC]  # [BC, N, D]
        xt_c = np.ascontiguousarray(xc.transpose(1, 2, 0)).astype(BF16)  # [N, D, BC]
        in_maps.append(
            {"xt": xt_c, "w": w_np, "b2": b2, "bleaf": bleaf, "b1": b1}
        )

    res = run_bass_kernel_spmd(
        nc, in_maps, core_ids=list(range(NCORES)), trace=bool(_compiled.get("trace"))
    )
    _compiled["last_res"] = res

    c_full = np.empty((B, D), np.float32)
    h_full = np.empty((B, D), np.float32)
    for c in range(NCORES):
        c_full[c * BC : (c + 1) * BC] = res.results[c]["c0t"].T
        h_full[c * BC : (c + 1) * BC] = res.results[c]["h0t"].T
    return c_full, h_full


# revision 15
# speedup vs baseline: 1.8163x; 1.8163x over previous
"""ConstituencyTreeLSTM Trainium2 kernel.

Strategy:
  - Data-parallel over the B=256 batch across 8 NeuronCores (32 rows/core).
  - The tree is a complete heap (node i has children 2i+1, 2i+2), so the
    sequential scan is reorganized into level-parallel phases:
      leaves (nodes 128..255) -> node 127 -> level 6 (63..126) -> ... -> root.
  - Everything on-device lives in a "feature-on-partitions, (node,batch) rows
    on free axis" layout, so matmul outputs (PSUM, [out_dim, rows]) are already
    in the layout needed to feed the next level's matmul. No transposes.
  - One fused bf16 weight matrix W_big [1536, 2560]:
      rows:  0:512 x | 512:1024 hL | 1024:1536 hR
      cols:  0:1536 iou | 1536:2048 fL-pre | 2048:2560 fR-pre
    Zero blocks (hR->fL, hL->fR) are skipped; only the 208 used 128x128
    blocks are stored (packed).
  - h of every level lives in SBUF level tiles; parents read children h via
    stride-2 node slices directly (no DRAM roundtrip on the critical path).
  - c goes through DRAM (CL/CR, parity-split by parent index) - it is only
    needed by the cheap elementwise stage, late in each chunk.
  - Per-node-type biases (2-child / leaf / 1-child) folded host-side and
    applied inside the PSUM-evacuating activation (sigmoid/tanh).
"""

import sys

sys.path.insert(0, "/opt/trn_rl_repo")

import numpy as np
import ml_dtypes

import concourse.bass as bass  # noqa: F401
import concourse.mybir as mybir
import concourse.tile as tile
from concourse import bacc
from concourse.bass_utils import run_bass_kernel_spmd

BF16 = ml_dtypes.bfloat16
NCORES = 8
B, N, D = 256, 256, 512
BC = B // NCORES  # batch rows per core
KT_X, KT_HL, KT_HR = range(0, 4), range(4, 8), range(8, 12)
NJ = 20  # output j-tiles: 12 iou + 4 fL + 4 fR

_compiled = {}


def _used_kts(j, has_l=True, has_r=True):
    if j < 12:
        kts = list(KT_X) + (list(KT_HL) if has_l else []) + (list(KT_HR) if has_r else [])
    elif j < 16:
        kts = list(KT_X) + list(KT_HL)
    else:
        kts = list(KT_X) + list(KT_HR)
    return kts


# packed weight-block index: only (kt, j) pairs with nonzero weight blocks
W_BLOCKS = [(kt, j) for j in range(NJ) for kt in _used_kts(j)]
W_IDX = {p: i for i, p in enumerate(W_BLOCKS)}
NW = len(W_BLOCKS)  # 208


def _build_bass(reps=1, kts_limit=None, skip_ew=False):
    nc = bacc.Bacc("TRN2", target_bir_lowering=False, debug=False, num_devices=NCORES)

    f32 = mybir.dt.float32
    bf16 = mybir.dt.bfloat16

    xt = nc.dram_tensor("xt", [N, D, BC], bf16, kind="ExternalInput")
    w = nc.dram_tensor("w", [NW, 128, 128], bf16, kind="ExternalInput")
    b2_d = nc.dram_tensor("b2", [128, NJ], f32, kind="ExternalInput")
    bleaf_d = nc.dram_tensor("bleaf", [128, NJ], f32, kind="ExternalInput")
    b1_d = nc.dram_tensor("b1", [128, NJ], f32, kind="ExternalInput")

    # children c keyed by parent index t: CL[t] = c(2t+1), CR[t] = c(2t+2)
    CL = nc.dram_tensor("CLbuf", [128, D, BC], bf16)
    CR = nc.dram_tensor("CRbuf", [128, D, BC], bf16)

    c0t = nc.dram_tensor("c0t", [D, BC], f32, kind="ExternalOutput")
    h0t = nc.dram_tensor("h0t", [D, BC], f32, kind="ExternalOutput")

    # all views are [partition, node, ktile, batch]; (node, ktile) merge on DMA
    xt_r = xt.ap().rearrange("n (kt p) b -> p n kt b", p=128)
    CL_r = CL.ap().rearrange("t (kt p) b -> p t kt b", p=128)
    CR_r = CR.ap().rearrange("t (kt p) b -> p t kt b", p=128)
    c0t_r = c0t.ap().rearrange("(kt p) b -> p kt b", p=128)
    h0t_r = h0t.ap().rearrange("(kt p) b -> p kt b", p=128)

    with tile.TileContext(nc) as tc:
        import contextlib

        ctx = contextlib.ExitStack()
        with ctx:
            wpool = ctx.enter_context(tc.tile_pool(name="wpool", bufs=1))
            hpool = ctx.enter_context(tc.tile_pool(name="hpool", bufs=1))
            inpool = ctx.enter_context(tc.tile_pool(name="inpool", bufs=2))
            gpool = ctx.enter_context(tc.tile_pool(name="gpool", bufs=2))
            epool = ctx.enter_context(tc.tile_pool(name="epool", bufs=2))
            pspool = ctx.enter_context(tc.tile_pool(name="ps", bufs=8, space="PSUM"))

            w_sb = wpool.tile([128, NW, 128], bf16)
            nc.sync.dma_start(out=w_sb[:], in_=w.ap().rearrange("blk p c -> p blk c"))
            b2_sb = wpool.tile([128, NJ], f32, name="b2sb")
            bleaf_sb = wpool.tile([128, NJ], f32, name="bleafsb")
            b1_sb = wpool.tile([128, NJ], f32, name="b1sb")
            nc.sync.dma_start(out=b2_sb[:], in_=b2_d.ap()[:])
            nc.sync.dma_start(out=bleaf_sb[:], in_=bleaf_d.ap()[:])
            nc.sync.dma_start(out=b1_sb[:], in_=b1_d.ap()[:])

            def process(
                nodes,
                has_l,
                has_r,
                bias_sb,
                child_h,  # (tile, base_node) or None
                out_h,  # (tile, base_node) or None (root)
            ):
                """Compute (c, h) for `nodes` (a range), all at the same depth."""
                to_out = out_h is None
                for a in range(nodes.start, nodes.stop, 16):
                    b_ = min(a + 16, nodes.stop)
                    k = b_ - a  # nodes in this chunk
                    dt_g = f32 if to_out else bf16

                    xt_t = inpool.tile([128, k, 4, BC], bf16, name="xt_t")
                    nc.sync.dma_start(out=xt_t[:], in_=xt_r[:, a:b_, :, :])
                    if has_l:
                        cl_t = inpool.tile([128, k, 4, BC], bf16, name="cl_t")
                        nc.sync.dma_start(out=cl_t[:], in_=CL_r[:, a:b_, :, :])
                    if has_r:
                        cr_t = inpool.tile([128, k, 4, BC], bf16, name="cr_t")
                        nc.sync.dma_start(out=cr_t[:], in_=CR_r[:, a:b_, :, :])
                    if child_h is not None:
                        ch_t, ch_base = child_h
                        sl0 = 2 * a + 1 - ch_base

                        def child_slice(kt, off):
                            s0 = sl0 + off
                            if k == 1:
                                return ch_t[:, s0 : s0 + 1, kt, :]
                            return ch_t[:, s0 : s0 + 2 * k - 1 : 2, kt, :]

                    g_i = gpool.tile([128, k, 4, BC], dt_g, name="g_i")
                    g_o = gpool.tile([128, k, 4, BC], dt_g, name="g_o")
                    g_u = gpool.tile([128, k, 4, BC], dt_g, name="g_u")
                    if has_l:
                        g_fl = gpool.tile([128, k, 4, BC], dt_g, name="g_fl")
                    if has_r:
                        g_fr = gpool.tile([128, k, 4, BC], dt_g, name="g_fr")

                    js = list(range(12))
                    if has_l:
                        js += list(range(12, 16))
                    if has_r:
                        js += list(range(16, 20))

                    for j in js:
                        kts = _used_kts(j, has_l, has_r)
                        if kts_limit:
                            kts = kts[:kts_limit]

                        ps = pspool.tile([128, k, BC], f32, name="ps")
                        for i, kt in enumerate(kts):
                            if kt < 4:
                                rhs = xt_t[:, :, kt, :]
                            elif kt < 8:
                                rhs = child_slice(kt - 4, 0)
                            else:
                                rhs = child_slice(kt - 8, 1)
                            nc.tensor.matmul(
                                ps[:],
                                w_sb[:, W_IDX[(kt, j)], :],
                                rhs,
                                start=(i == 0),
                                stop=(i == len(kts) - 1),
                            )
                        func = (
                            mybir.ActivationFunctionType.Tanh
                            if 8 <= j < 12
                            else mybir.ActivationFunctionType.Sigmoid
                        )
                        if j < 4:
                            dst = g_i[:, :, j, :]
                        elif j < 8:
                            dst = g_o[:, :, j - 4, :]
                        elif j < 12:
                            dst = g_u[:, :, j - 8, :]
                        elif j < 16:
                            dst = g_fl[:, :, j - 12, :]
                        else:
                            dst = g_fr[:, :, j - 16, :]
                        nc.scalar.activation(
                            out=dst,
                            in_=ps[:],
                            func=func,
                            bias=bias_sb[:, j : j + 1],
                            scale=1.0,
                        )

                    if skip_ew:
                        continue

                    # c = i*u (+ fl*cl) (+ fr*cr);  h = o * tanh(c)
                    c_t = epool.tile([128, k, 4, BC], dt_g, name="c_t")
                    nc.vector.tensor_mul(c_t[:], g_i[:], g_u[:])
                    if has_l:
                        m2 = epool.tile([128, k, 4, BC], dt_g, name="mt")
                        nc.vector.tensor_mul(m2[:], g_fl[:], cl_t[:])
                        nc.vector.tensor_add(c_t[:], c_t[:], m2[:])
                    if has_r:
                        m3 = epool.tile([128, k, 4, BC], dt_g, name="mt")
                        nc.vector.tensor_mul(m3[:], g_fr[:], cr_t[:])
                        nc.vector.tensor_add(c_t[:], c_t[:], m3[:])
                    tc_t = epool.tile([128, k, 4, BC], dt_g, name="tc_t")
                    nc.scalar.activation(
                        out=tc_t[:], in_=c_t[:], func=mybir.ActivationFunctionType.Tanh
                    )

                    if to_out:
                        h_t = epool.tile([128, k, 4, BC], dt_g, name="h_t")
                        nc.vector.tensor_mul(h_t[:], g_o[:], tc_t[:])
                        nc.sync.dma_start(out=c0t_r[:], in_=c_t[:, 0, :, :])
                        nc.sync.dma_start(out=h0t_r[:], in_=h_t[:, 0, :, :])
                    else:
                        oh_t, oh_base = out_h
                        nc.vector.tensor_mul(
                            oh_t[:, a - oh_base : b_ - oh_base, :, :], g_o[:], tc_t[:]
                        )
                        # c of node t -> CL[(t-1)//2] if t odd else CR[t//2 - 1]
                        odd0 = 0 if a % 2 == 1 else 1
                        even0 = 1 - odd0
                        odds = range(a + odd0, b_, 2)
                        evens = range(a + even0, b_, 2)
                        for kt in range(4):
                            if len(odds):
                                lo = (odds[0] - 1) // 2
                                nc.sync.dma_start(
                                    out=CL_r[:, lo : lo + len(odds), kt, :],
                                    in_=c_t[:, odd0::2, kt, :],
                                )
                            if len(evens):
                                ro = evens[0] // 2 - 1
                                nc.sync.dma_start(
                                    out=CR_r[:, ro : ro + len(evens), kt, :],
                                    in_=c_t[:, even0::2, kt, :],
                                )

            for _rep in range(reps):
                # per-level h tiles (SBUF-resident)
                leafc_h = hpool.tile([128, 129, 4, BC], bf16, name="h_leafc")
                lvl_h = {7: (leafc_h, 127)}
                for lvl in range(6, 0, -1):
                    t = hpool.tile([128, 2**lvl, 4, BC], bf16, name=f"h_{lvl}")
                    lvl_h[lvl] = (t, 2**lvl - 1)

                # leaves: nodes 128..255 (no children)
                process(range(128, 256), False, False, bleaf_sb, None, lvl_h[7])
                # node 127: left child only (node 255, leafc slot 128)
                process(range(127, 128), True, False, b1_sb, lvl_h[7], lvl_h[7])
                # levels 6..1: two children each
                for lvl in range(6, 0, -1):
                    process(
                        range(2**lvl - 1, 2 ** (lvl + 1) - 1),
                        True,
                        True,
                        b2_sb,
                        lvl_h[lvl + 1] if lvl < 6 else lvl_h[7],
                        lvl_h[lvl],
                    )
                # root
                process(range(0, 1), True, True, b2_sb, lvl_h[1], None)

    nc.compile()
    return nc


def _expected_tree():
    left = np.array([2 * i + 1 if 2 * i + 1 < N else 0 for i in range(N)], np.int32)
    right = np.array([2 * i + 2 if 2 * i + 2 < N else 0 for i in range(N)], np.int32)
    nch = np.array(
        [int(2 * i + 1 < N) + int(2 * i + 2 < N) for i in range(N)], np.int32
    )
    return left, right, nch


def pack_w(W_ioux, W_fx, W_iouhL, W_fhL, W_iouhR, W_fhR):
    w_big = np.zeros((1536, 2560), np.float32)
    w_big[0:512, 0:1536] = np.asarray(W_ioux, np.float32).T
    w_big[0:512, 1536:2048] = np.asarray(W_fx, np.float32).T
    w_big[0:512, 2048:2560] = np.asarray(W_fx, np.float32).T
    w_big[512:1024, 0:1536] = np.asarray(W_iouhL, np.float32).T
    w_big[512:1024, 1536:2048] = np.asarray(W_fhL, np.float32).T
    w_big[1024:1536, 0:1536] = np.asarray(W_iouhR, np.float32).T
    w_big[1024:1536, 2048:2560] = np.asarray(W_fhR, np.float32).T
    w_np = np.empty((NW, 128, 128), np.float32)
    for i, (kt, j) in enumerate(W_BLOCKS):
        w_np[i] = w_big[kt * 128 : (kt + 1) * 128, j * 128 : (j + 1) * 128]
    return np.ascontiguousarray(w_np).astype(BF16)


def pack_biases(b_ioux, b_iouh, b_iouhL, b_iouhR, b_fx, b_fhL, b_fhR):
    def pack(vec):
        return np.ascontiguousarray(np.asarray(vec, np.float32).reshape(NJ, 128).T)

    z = np.zeros(512, np.float32)
    b2 = pack(np.concatenate([b_ioux + b_iouhL + b_iouhR, b_fx + b_fhL, b_fx + b_fhR]))
    bleaf = pack(np.concatenate([b_ioux + b_iouh, z, z]))
    b1 = pack(np.concatenate([b_ioux + b_iouhL, b_fx + b_fhL, z]))
    return b2, bleaf, b1


def kernel(
    inputs,
    W_ioux, b_ioux, W_iouh, b_iouh, W_iouhL, b_iouhL, W_iouhR, b_iouhR,
    W_fx, b_fx, W_fh, b_fh, W_fhL, b_fhL, W_fhR, b_fhR,
    left_idx, right_idx, num_children,
):
    el, er, en = _expected_tree()
    assert np.array_equal(np.asarray(left_idx), el), "unexpected tree structure"
    assert np.array_equal(np.asarray(right_idx), er), "unexpected tree structure"
    assert np.array_equal(np.asarray(num_children), en), "unexpected tree structure"

    inputs = np.asarray(inputs, np.float32)

    w_np = pack_w(W_ioux, W_fx, W_iouhL, W_fhL, W_iouhR, W_fhR)
    b_args = [
        np.asarray(v, np.float32)
        for v in (b_ioux, b_iouh, b_iouhL, b_iouhR, b_fx, b_fhL, b_fhR)
    ]
    b2, bleaf, b1 = pack_biases(*b_args)

    if "nc" not in _compiled:
        _compiled["nc"] = _build_bass()
    nc = _compiled["nc"]

    in_maps = []
    for c in range(NCORES):
        xc = inputs[c * BC : (c + 1) * BC]  # [BC, N, D]
        xt_c = np.ascontiguousarray(xc.transpose(1, 2, 0)).astype(BF16)  # [N, D, BC]
        in_maps.append({"xt": xt_c, "w": w_np, "b2": b2, "bleaf": bleaf, "b1": b1})

    res = run_bass_kernel_spmd(
        nc, in_maps, core_ids=list(range(NCORES)), trace=bool(_compiled.get("trace"))
    )
    _compiled["last_res"] = res

    c_full = np.empty((B, D), np.float32)
    h_full = np.empty((B, D), np.float32)
    for c in range(NCORES):
        c_full[c * BC : (c + 1) * BC] = res.results[c]["c0t"].T
        h_full[c * BC : (c + 1) * BC] = res.results[c]["h0t"].T
    return c_full, h_full
